# revision 4
# baseline (speedup 1.0000x reference)
"""MoE BERT block kernel for 8 Trainium2 NeuronCores.

Strategy: expert parallel. The router (gate matmul + softmax + top-2) is a
~134 MFLOP computation done on the host in float64 while sharding the inputs;
token dispatch by router assignment happens during the host-side shard step.
Each of the 8 cores owns one expert's FFN weights (SBUF-resident, bf16) and
runs the dense FFN over the tokens routed to it (padded to a fixed capacity),
which is >99.9% of the FLOPs. The host then scatter-adds `w * y` per token.

Device math per core (expert e), all tokens column-major (token = free dim):
    H^T = gelu(WupT^T @ X^T + bup)      # [4096, CAP]  bf16, f32 accum
    Y^T = WdownT^T @ H^T + bdown        # [1024, CAP]  f32
"""

import os

os.environ.setdefault("MYCRO_LOCAL_CACHE", "1")

import numpy as np
import ml_dtypes

import concourse.bass as bass
import concourse.bacc as bacc
import concourse.mybir as mybir
import concourse.tile as tile
from concourse.bass_utils import run_bass_kernel_spmd

NUM_EXPERTS = 8
TOP_K = 2
H = 1024
I = 4096
P = 128
CAP = 2304  # per-expert token capacity (max observed load 2161, mean 2048)
TOKEN_TILES = [512, 512, 512, 512, 256]
assert sum(TOKEN_TILES) == CAP

BF16 = mybir.dt.bfloat16
F32 = mybir.dt.float32

_compiled = None  # (nc,) cache — build the Bass program once per process
last_results = None  # BassKernelResults of the most recent run (for profiling)


def _build_program():
    nc = bacc.Bacc("TRN2", target_bir_lowering=False)

    xt = nc.dram_tensor("xt", [H, CAP], BF16, kind="ExternalInput")
    wup_t = nc.dram_tensor("wup_t", [H, I], BF16, kind="ExternalInput")
    wdn_t = nc.dram_tensor("wdn_t", [I, H], BF16, kind="ExternalInput")
    bup = nc.dram_tensor("bup", [I], F32, kind="ExternalInput")
    bdn = nc.dram_tensor("bdn", [H], F32, kind="ExternalInput")
    yt = nc.dram_tensor("yt", [H, CAP], F32, kind="ExternalOutput")

    KO = H // P  # 8 contraction tiles for the up matmul
    IO = I // P  # 32 inter tiles (psum partition tiles up / contraction down)
    HO = H // P  # 8 output tiles for the down matmul

    with tile.TileContext(nc) as tc:
        with (
            tc.tile_pool(name="weights", bufs=1) as wpool,
            tc.tile_pool(name="xin", bufs=2) as xpool,
            tc.tile_pool(name="hmid", bufs=1) as hpool,
            tc.tile_pool(name="yout", bufs=4) as ypool,
            tc.tile_pool(name="psum_up", bufs=4, space="PSUM") as pu,
            tc.tile_pool(name="psum_dn", bufs=4, space="PSUM") as pd,
        ):
            # Weights resident in SBUF for the whole kernel (128 KiB/partition).
            wup_sb = wpool.tile([P, KO, I], BF16, tag="wup")
            nc.sync.dma_start(wup_sb[:], wup_t.ap().rearrange("(ko p) i -> p ko i", p=P))
            wdn_sb = wpool.tile([P, IO, H], BF16, tag="wdn")
            nc.sync.dma_start(wdn_sb[:], wdn_t.ap().rearrange("(io p) h -> p io h", p=P))
            bup_sb = wpool.tile([P, IO], F32, tag="bup")
            nc.sync.dma_start(bup_sb[:], bup.ap().rearrange("(io p) -> p io", p=P))
            bdn_sb = wpool.tile([P, HO], F32, tag="bdn")
            nc.sync.dma_start(bdn_sb[:], bdn.ap().rearrange("(ho p) -> p ho", p=P))

            xt_r = xt.ap().rearrange("(ko p) t -> p ko t", p=P)
            yt_r = yt.ap().rearrange("(ho p) t -> p ho t", p=P)

            off = 0
            for ntok in TOKEN_TILES:
                x_sb = xpool.tile([P, KO, TOKEN_TILES[0]], BF16, tag="x")
                nc.sync.dma_start(
                    x_sb[:, :, :ntok], xt_r[:, :, off : off + ntok]
                )

                # Up-projection + exact (erf) GELU: H^T tile [4096, ntok].
                h_sb = hpool.tile([P, IO, TOKEN_TILES[0]], BF16, tag="h")
                for io in range(IO):
                    ps = pu.tile([P, TOKEN_TILES[0]], F32, tag="pu")
                    for ko in range(KO):
                        nc.tensor.matmul(
                            ps[:, :ntok],
                            lhsT=wup_sb[:, ko, io * P : (io + 1) * P],
                            rhs=x_sb[:, ko, :ntok],
                            start=(ko == 0),
                            stop=(ko == KO - 1),
                        )
                    nc.scalar.activation(
                        h_sb[:, io, :ntok],
                        ps[:, :ntok],
                        mybir.ActivationFunctionType.Gelu,
                        bias=bup_sb[:, io : io + 1],
                        scale=1.0,
                    )

                # Down-projection + bias: Y^T tile [1024, ntok] f32.
                for ho in range(HO):
                    ps2 = pd.tile([P, TOKEN_TILES[0]], F32, tag="pd")
                    for io in range(IO):
                        nc.tensor.matmul(
                            ps2[:, :ntok],
                            lhsT=wdn_sb[:, io, ho * P : (ho + 1) * P],
                            rhs=h_sb[:, io, :ntok],
                            start=(io == 0),
                            stop=(io == IO - 1),
                        )
                    y_sb = ypool.tile([P, TOKEN_TILES[0]], F32, tag="y")
                    nc.vector.tensor_scalar_add(
                        y_sb[:, :ntok], ps2[:, :ntok], bdn_sb[:, ho : ho + 1]
                    )
                    nc.sync.dma_start(
                        yt_r[:, ho, off : off + ntok], y_sb[:, :ntok]
                    )
                off += ntok

    nc.compile()
    return nc


def _get_program():
    global _compiled
    if _compiled is None:
        _compiled = _build_program()
    return _compiled


def _route(X64, Wg64):
    """Replicates the reference router: softmax over gate logits, top-2."""
    T = X64.shape[0]
    logits = X64 @ Wg64.T  # [T, E]
    logits -= logits.max(axis=-1, keepdims=True)
    p = np.exp(logits)
    p /= p.sum(axis=-1, keepdims=True)
    i1 = np.argmax(p, axis=-1)
    rows = np.arange(T)
    w1 = p[rows, i1]
    p2 = p.copy()
    p2[rows, i1] = -1.0
    i2 = np.argmax(p2, axis=-1)
    w2 = p[rows, i2]
    return i1, w1, i2, w2


def kernel(hidden_states, Wg, Wup, bup, Wdown, bdown):
    global last_results
    hidden_states = np.asarray(hidden_states)
    orig_shape = hidden_states.shape
    X = np.ascontiguousarray(hidden_states, dtype=np.float32).reshape(-1, H)
    T = X.shape[0]
    Wg = np.asarray(Wg, dtype=np.float32)
    Wup = np.asarray(Wup, dtype=np.float32)
    bup = np.asarray(bup, dtype=np.float32)
    Wdown = np.asarray(Wdown, dtype=np.float32)
    bdown = np.asarray(bdown, dtype=np.float32)

    # --- Router on host (float64 for a faithful top-2 ordering) ---
    i1, w1, i2, w2 = _route(X.astype(np.float64), Wg.astype(np.float64))

    # --- Dispatch: gather each expert's tokens, pad to CAP ---
    Xb = X.astype(ml_dtypes.bfloat16)
    in_maps = []
    meta = []
    for e in range(NUM_EXPERTS):
        sel1 = np.nonzero(i1 == e)[0]
        sel2 = np.nonzero(i2 == e)[0]
        idx = np.concatenate([sel1, sel2])
        wts = np.concatenate([w1[sel1], w2[sel2]])
        n = idx.size
        if n > CAP:
            raise RuntimeError(f"expert {e} token count {n} exceeds capacity {CAP}")
        idx_pad = np.concatenate([idx, np.zeros(CAP - n, dtype=idx.dtype)])
        in_maps.append(
            {
                "xt": np.ascontiguousarray(Xb[idx_pad].T),
                "wup_t": np.ascontiguousarray(Wup[e].astype(ml_dtypes.bfloat16).T),
                "wdn_t": np.ascontiguousarray(Wdown[e].astype(ml_dtypes.bfloat16).T),
                "bup": np.ascontiguousarray(bup[e]),
                "bdn": np.ascontiguousarray(bdown[e]),
            }
        )
        meta.append((idx, wts))

    # --- Run the Bass kernel on all 8 cores ---
    nc = _get_program()
    last_results = run_bass_kernel_spmd(nc, in_maps, core_ids=list(range(8)))

    # --- Combine: out[token] += w * y ---
    out = np.zeros((T, H), dtype=np.float32)
    for e in range(NUM_EXPERTS):
        idx, wts = meta[e]
        yt_full = np.asarray(last_results.results[e]["yt"])  # [H, CAP] f32
        Y = yt_full.T[: idx.size]  # [n, H]
        out[idx] += wts[:, None].astype(np.float32) * Y
    return out.reshape(orig_shape)


# revision 11
# speedup vs baseline: 1.1123x; 1.1123x over previous
"""MoE BERT block kernel for 8 Trainium2 NeuronCores.

Strategy: expert parallel. The router (gate matmul + softmax + top-2) is a
~134 MFLOP computation done on the host in float64 while sharding the inputs;
token dispatch by router assignment happens during the host-side shard step.
Each of the 8 cores owns one expert's FFN weights (SBUF-resident, bf16) and
runs the dense FFN over the tokens routed to it (padded to a fixed capacity),
which is >99.9% of the FLOPs. The host then scatter-adds `w * y` per token.

Device math per core (expert e), all tokens column-major (token = free dim):
    H^T = gelu(WupT^T @ X^T + bup)      # [4096, CAP]  bf16, f32 accum
    Y^T = WdownT^T @ H^T + bdown        # [1024, CAP]  f32
"""

import os

os.environ.setdefault("MYCRO_LOCAL_CACHE", "1")

import numpy as np
import ml_dtypes

import concourse.bass as bass
import concourse.bacc as bacc
import concourse.mybir as mybir
import concourse.tile as tile
from concourse.bass_utils import run_bass_kernel_spmd

NUM_EXPERTS = 8
TOP_K = 2
H = 1024
I = 4096
P = 128
CAP = 2176  # per-expert token capacity (max observed load 2161, mean 2048);
# tokens beyond CAP (never expected for the reference inputs) fall back to a
# host-side numpy computation, so correctness never depends on this margin.
TOKEN_TILES = [512, 512, 512, 512, 128]
assert sum(TOKEN_TILES) == CAP

BF16 = mybir.dt.bfloat16
F32 = mybir.dt.float32

_compiled = None  # (nc,) cache — build the Bass program once per process
last_results = None  # BassKernelResults of the most recent run (for profiling)


def _build_program():
    nc = bacc.Bacc("TRN2", target_bir_lowering=False)

    xt = nc.dram_tensor("xt", [H, CAP], BF16, kind="ExternalInput")
    wup_t = nc.dram_tensor("wup_t", [H, I], BF16, kind="ExternalInput")
    wdn_t = nc.dram_tensor("wdn_t", [I, H], BF16, kind="ExternalInput")
    bup = nc.dram_tensor("bup", [I], F32, kind="ExternalInput")
    bdn = nc.dram_tensor("bdn", [H], F32, kind="ExternalInput")
    yt = nc.dram_tensor("yt", [H, CAP], F32, kind="ExternalOutput")

    KO = H // P  # 8 contraction tiles for the up matmul
    IO = I // P  # 32 inter tiles (psum partition tiles up / contraction down)
    HO = H // P  # 8 output tiles for the down matmul

    UPB = 4  # psum banks per up-projection block
    DNB = 4  # psum banks per down-projection block

    with tile.TileContext(nc) as tc:
        with (
            tc.tile_pool(name="weights", bufs=1) as wpool,
            tc.tile_pool(name="xin", bufs=2) as xpool,
            tc.tile_pool(name="hmid", bufs=1) as hpool,
            tc.tile_pool(name="yout", bufs=4) as ypool,
            tc.tile_pool(name="psum_up", bufs=UPB, space="PSUM") as pu,
            tc.tile_pool(name="psum_dn", bufs=DNB, space="PSUM") as pd,
        ):
            xt_r = xt.ap().rearrange("(ko p) t -> p ko t", p=P)
            yt_r = yt.ap().rearrange("(ho p) t -> p ho t", p=P)
            wup_r = wup_t.ap().rearrange("(ko p) i -> p ko i", p=P)
            wdn_r = wdn_t.ap().rearrange("(io p) h -> p io h", p=P)

            # DMA issue order is chosen so compute can start early: the first
            # token tile, then the up weights (per-ko chunks, just-in-time for
            # the ko-outer first block), biases; the down weights stream in
            # per-io chunks interleaved with tile 0's up phase.
            x0_sb = xpool.tile([P, KO, TOKEN_TILES[0]], BF16, tag="x")
            nc.sync.dma_start(x0_sb[:], xt_r[:, :, 0 : TOKEN_TILES[0]])

            wup_sb = wpool.tile([P, KO, I], BF16, tag="wup")
            for ko in range(KO):
                nc.sync.dma_start(wup_sb[:, ko], wup_r[:, ko])
            bup_sb = wpool.tile([P, IO], F32, tag="bup")
            nc.sync.dma_start(bup_sb[:], bup.ap().rearrange("(io p) -> p io", p=P))
            bdn_sb = wpool.tile([P, HO], F32, tag="bdn")
            nc.sync.dma_start(bdn_sb[:], bdn.ap().rearrange("(ho p) -> p ho", p=P))
            wdn_sb = wpool.tile([P, IO, H], BF16, tag="wdn")

            off = 0
            for t, ntok in enumerate(TOKEN_TILES):
                if t == 0:
                    x_sb = x0_sb
                else:
                    x_sb = xpool.tile([P, KO, TOKEN_TILES[0]], BF16, tag="x")
                    nc.sync.dma_start(
                        x_sb[:, :, :ntok], xt_r[:, :, off : off + ntok]
                    )

                # Up-projection + exact (erf) GELU: H^T tile [4096, ntok].
                # Blocks of UPB psum banks, contraction (ko) outer within a
                # block so the first matmul only needs one weight chunk.
                h_sb = hpool.tile([P, IO, TOKEN_TILES[0]], BF16, tag="h")
                for blk in range(IO // UPB):
                    pss = [pu.tile([P, TOKEN_TILES[0]], F32, tag="pu", name=f"pu{j}") for j in range(UPB)]
                    for ko in range(KO):
                        for j in range(UPB):
                            io = blk * UPB + j
                            nc.tensor.matmul(
                                pss[j][:, :ntok],
                                lhsT=wup_sb[:, ko, io * P : (io + 1) * P],
                                rhs=x_sb[:, ko, :ntok],
                                start=(ko == 0),
                                stop=(ko == KO - 1),
                            )
                    for j in range(UPB):
                        io = blk * UPB + j
                        nc.scalar.activation(
                            h_sb[:, io, :ntok],
                            pss[j][:, :ntok],
                            mybir.ActivationFunctionType.Gelu,
                            bias=bup_sb[:, io : io + 1],
                            scale=1.0,
                        )
                    if t == 0:
                        # Stream the down weights while tile 0's up phase runs.
                        for io in range(blk * UPB, (blk + 1) * UPB):
                            nc.sync.dma_start(wdn_sb[:, io], wdn_r[:, io])

                # Down-projection + bias: Y^T tile [1024, ntok] f32.
                # Contraction (io) outer within a DNB-bank block: the first
                # matmuls only need h[:, 0] (ready early in the up phase).
                for blk in range(HO // DNB):
                    ps2 = [pd.tile([P, TOKEN_TILES[0]], F32, tag="pd", name=f"pd{j}") for j in range(DNB)]
                    for io in range(IO):
                        for j in range(DNB):
                            ho = blk * DNB + j
                            nc.tensor.matmul(
                                ps2[j][:, :ntok],
                                lhsT=wdn_sb[:, io, ho * P : (ho + 1) * P],
                                rhs=h_sb[:, io, :ntok],
                                start=(io == 0),
                                stop=(io == IO - 1),
                            )
                    for j in range(DNB):
                        ho = blk * DNB + j
                        y_sb = ypool.tile([P, TOKEN_TILES[0]], F32, tag="y")
                        nc.vector.tensor_scalar_add(
                            y_sb[:, :ntok], ps2[j][:, :ntok], bdn_sb[:, ho : ho + 1]
                        )
                        nc.sync.dma_start(
                            yt_r[:, ho, off : off + ntok], y_sb[:, :ntok]
                        )
                off += ntok

    nc.compile()
    return nc


def _get_program():
    global _compiled
    if _compiled is None:
        _compiled = _build_program()
    return _compiled


def _route(X64, Wg64):
    """Replicates the reference router: softmax over gate logits, top-2."""
    T = X64.shape[0]
    logits = X64 @ Wg64.T  # [T, E]
    logits -= logits.max(axis=-1, keepdims=True)
    p = np.exp(logits)
    p /= p.sum(axis=-1, keepdims=True)
    i1 = np.argmax(p, axis=-1)
    rows = np.arange(T)
    w1 = p[rows, i1]
    p2 = p.copy()
    p2[rows, i1] = -1.0
    i2 = np.argmax(p2, axis=-1)
    w2 = p[rows, i2]
    return i1, w1, i2, w2


def kernel(hidden_states, Wg, Wup, bup, Wdown, bdown):
    global last_results
    hidden_states = np.asarray(hidden_states)
    orig_shape = hidden_states.shape
    X = np.ascontiguousarray(hidden_states, dtype=np.float32).reshape(-1, H)
    T = X.shape[0]
    Wg = np.asarray(Wg, dtype=np.float32)
    Wup = np.asarray(Wup, dtype=np.float32)
    bup = np.asarray(bup, dtype=np.float32)
    Wdown = np.asarray(Wdown, dtype=np.float32)
    bdown = np.asarray(bdown, dtype=np.float32)

    # --- Router on host (float64 for a faithful top-2 ordering) ---
    i1, w1, i2, w2 = _route(X.astype(np.float64), Wg.astype(np.float64))

    # --- Dispatch: gather each expert's tokens, pad to CAP ---
    Xb = X.astype(ml_dtypes.bfloat16)
    in_maps = []
    meta = []
    for e in range(NUM_EXPERTS):
        sel1 = np.nonzero(i1 == e)[0]
        sel2 = np.nonzero(i2 == e)[0]
        idx = np.concatenate([sel1, sel2])
        wts = np.concatenate([w1[sel1], w2[sel2]])
        n = idx.size
        overflow = None
        if n > CAP:
            # Never expected for the reference inputs (max load 2161); kept as
            # a correctness safety net: spill tokens are computed on the host.
            overflow = (idx[CAP:], wts[CAP:])
            idx, wts = idx[:CAP], wts[:CAP]
            n = CAP
        idx_pad = np.concatenate([idx, np.zeros(CAP - n, dtype=idx.dtype)])
        in_maps.append(
            {
                "xt": np.ascontiguousarray(Xb[idx_pad].T),
                "wup_t": np.ascontiguousarray(Wup[e].astype(ml_dtypes.bfloat16).T),
                "wdn_t": np.ascontiguousarray(Wdown[e].astype(ml_dtypes.bfloat16).T),
                "bup": np.ascontiguousarray(bup[e]),
                "bdn": np.ascontiguousarray(bdown[e]),
            }
        )
        meta.append((idx, wts, overflow))

    # --- Run the Bass kernel on all 8 cores ---
    nc = _get_program()
    last_results = run_bass_kernel_spmd(nc, in_maps, core_ids=list(range(8)))

    # --- Combine: out[token] += w * y ---
    out = np.zeros((T, H), dtype=np.float32)
    for e in range(NUM_EXPERTS):
        idx, wts, overflow = meta[e]
        yt_full = np.asarray(last_results.results[e]["yt"])  # [H, CAP] f32
        Y = yt_full.T[: idx.size]  # [n, H]
        out[idx] += wts[:, None].astype(np.float32) * Y
        if overflow is not None:
            oidx, owts = overflow
            from scipy.special import erf

            xo = X[oidx]
            h_in = xo @ Wup[e].T + bup[e]
            h = 0.5 * h_in * (1.0 + erf(h_in / np.sqrt(2.0)))
            yo = h @ Wdown[e].T + bdown[e]
            out[oidx] += owts[:, None].astype(np.float32) * yo
    return out.reshape(orig_shape)
